# revision 57
# baseline (speedup 1.0000x reference)
"""Trainium2 Bass kernel for nn_Decoder (LAS-style attention decoder).

Data-parallel over batch: 64 batches -> 8 cores x 8 batches. Per core, the
full 256-step recurrence runs on-chip: key/value/weights live in SBUF (bf16),
LSTM gates / attention are PE matmuls with small stationaries and streamed
rhs, softmax exp on ACT, elementwise on DVE, layout flips via DMA transpose.

Host (numpy) does all data layout: transposed/prescaled weight tables,
one-hot id table, zero-padded key/value tails (mask), value ones-column
(softmax denominator), initial context. The device program itself is
input-independent.

Math notes:
 - sigmoid(x) = 0.5*tanh(0.5x)+0.5; the cell-input gate g needs tanh(g), so
   its weight columns are pre-scaled by 2 on host and ONE ACT tanh with
   scale=0.5 covers all four gates.
 - cell state c is stored doubled (cd = 2c) so the 0.5 factors fold into
   single scalar_tensor_tensor ops and the ACT tanh scale.
 - softmax: energies are bounded (|q|<=1), so exp without max-subtraction is
   safe; masked t positions have key/value zeroed on host, the value ones-
   column accumulates the valid-only denominator during the context matmul.
"""
import os
import sys
import math
from contextlib import ExitStack

import numpy as np
import ml_dtypes

for _p in ("/root/.axon_site", "/root/.axon_site/_ro/trn_rl_repo",
           "/root/.axon_site/_ro/pypackages"):
    if _p not in sys.path and os.path.isdir(_p):
        sys.path.append(_p)

import concourse.bass as bass
import concourse.tile as tile
from concourse import mybir
from concourse._compat import not_none as nn

F32 = mybir.dt.float32
BF16 = mybir.dt.bfloat16
AF = mybir.ActivationFunctionType
OP = mybir.AluOpType

# ---------------------------------------------------------------------------
# Workaround: this walrus build rejects multi-wait Drain instructions
# (setupSyncWait TPB_CTRL "Too many sync wait commands"). Split the Tile
# kernel-tail drain into single-wait drains.
_orig_drain_and_barrier = tile.TileContext._drain_and_barrier

def _patched_drain_and_barrier(self, tick_clock, wait_clock):
    from concourse.tile import ScopedClock
    drain_inst = self.nc.sync.drain()
    wait_clock.add_sem_waits(drain_inst.ins,
                             ScopedClock({None: tick_clock.global_clock}))
    si = drain_inst.ins.sync_info
    waits = list(si.on_wait or []) if si is not None else []
    if len(waits) > 1:
        bb = nn(self.nc.cur_bb).bb
        drain_inst.ins.sync_info.on_wait = waits[:1]
        idx = bb.instructions.index(drain_inst.ins)
        for w in waits[1:]:
            extra = self.nc.sync.drain()
            extra.ins.sync_info = mybir.SyncInfo(on_wait=[w], on_update=[])
            bb.instructions.remove(extra.ins)
            bb.instructions.insert(idx, extra.ins)
            idx += 1
    self.nc.all_engine_barrier()
    assert self.sems is not None
    popped = self.nc._tile_sem_poison_stack.pop()
    assert popped is self._sem_poison
    self.nc.clear_and_free_semaphores(list(self.sems.allocated().values()))
    self.nc.all_engine_barrier()

tile.TileContext._drain_and_barrier = _patched_drain_and_barrier


def split_multi_waits(nc):
    """This walrus build allows only one sync-wait per instruction in several
    ISA structs. Hoist all-but-one wait onto same-engine NOPs placed before
    the instruction (the engine queue blocks on them in order)."""
    n = 0
    for f in nc.m.functions:
        for bb in f.blocks:
            out = []
            for ins in bb.instructions:
                si = getattr(ins, "sync_info", None)
                waits = list(si.on_wait) if si is not None and si.on_wait else []
                if len(waits) > 1:
                    for w in waits[:-1]:
                        n += 1
                        nop = mybir.InstNoOp(
                            name=nc.get_next_instruction_name(),
                            engine=ins.engine, ins=[], outs=[],
                            sync_info=mybir.SyncInfo(on_wait=[w],
                                                     on_update=[]))
                        out.append(nop)
                    ins.sync_info.on_wait = waits[-1:]
                out.append(ins)
            bb.instructions[:] = out
    return n

# ---------------------------------------------------------------------------
# Problem constants (hardcoded per spec)
B, T, KV, E, H, V, L = 64, 2048, 128, 256, 512, 30, 256
NCORES = 8
BL = B // NCORES          # 8 batches per core
NCH = T // 128            # 16 t-chunks
SOS, EOS = 1, 0
SCALE = 1.0 / math.sqrt(KV)
GP = 16                   # padded gather partitions (dma transpose needs %16)


def _bf(x):
    return np.ascontiguousarray(np.asarray(x, np.float32).astype(ml_dtypes.bfloat16))


def _f32(x):
    return np.ascontiguousarray(np.asarray(x, np.float32))


def host_prep(key, value, encoder_len, y, emb_weight, W_ih1, W_hh1,
              W_ih2, W_hh2):
    """Build per-core device input tensors (all layout work on host)."""
    key = _f32(key).copy()
    value = _f32(value).copy()
    enc = np.asarray(encoder_len).astype(np.int64)
    y = np.asarray(y)
    emb_weight = _f32(emb_weight)
    W_ih1 = _f32(W_ih1); W_hh1 = _f32(W_hh1)
    W_ih2 = _f32(W_ih2); W_hh2 = _f32(W_hh2)

    # initial context: mean over FULL T of raw value (reference semantics)
    ctx0 = value.mean(axis=1)                      # (B, KV) fp32

    # gate g-slice prescale by 2 (tanh(g) = tanh(2g * 0.5))
    def g2(W, D):
        W = W.copy(); W[2 * D:3 * D, :] *= 2.0; return W

    EW = (emb_weight @ W_ih1[:, :E].T)             # (V, 4H)
    EW = g2(EW.T, H).T
    Wc1 = g2(W_ih1[:, E:], H)                      # (4H, KV)
    W_hh1p = g2(W_hh1, H)
    W_ih2p = g2(W_ih2, KV)
    W_hh2p = g2(W_hh2, KV)

    # weight streams, rhs layout [K-chunk partitions, free]
    Wr1T = np.transpose(W_hh1p.T.reshape(4, 128, 4 * H), (1, 0, 2))
    Wc1T = Wc1.T                                   # (128, 2048)
    Wih2T = np.transpose(W_ih2p.T.reshape(4, 128, 4 * KV), (1, 0, 2))
    Whh2T = W_hh2p.T                               # (128, 512)
    EmbOutT = np.transpose(emb_weight.T.reshape(2, 128, V), (1, 0, 2))

    ids = np.concatenate([np.full((B, 1), SOS, y.dtype), y[:, :-1]], axis=1)

    # zero mask tails (after ctx0 which uses raw value)
    for b in range(B):
        key[b, enc[b]:, :] = 0.0
        value[b, enc[b]:, :] = 0.0

    per_core = []
    for c in range(NCORES):
        sl = slice(c * BL, (c + 1) * BL)
        k_c = key[sl]
        v_c = value[sl]
        keyT = np.transpose(k_c, (2, 0, 1))        # (128, BL, T)
        v4 = v_c.reshape(BL, NCH, 128, KV)         # (b, c, p, kv)
        v_sb = np.transpose(v4, (2, 1, 0, 3))      # (p, c, b, kv)
        # softmax denominator correction: exp(0)=1 at masked t, so
        # valid_sum = full_sum - (T - len_b); batch 4j+r at energy row 32j+r
        tailT = np.zeros((48, 1), np.float32)
        for b in range(BL):
            tailT[32 * (b // 4) + (b % 4), 0] = float(T - int(enc[c * BL + b]))
        oh = np.zeros((V, L * BL), np.float32)
        for b in range(BL):
            oh[ids[c * BL + b], np.arange(L) * BL + b] = 1.0
        ctxT0 = ctx0[sl].T                         # (KV, BL)
        per_core.append(dict(
            keyT=_bf(keyT.reshape(128, BL * T)),
            v_sb=_bf(v_sb.reshape(128, NCH * BL * KV)),
            tailT=tailT,
            Wr1T=_bf(Wr1T.reshape(128, 16 * H)),
            Wc1T=_bf(Wc1T),
            EW=_bf(EW),
            Wih2T=_bf(Wih2T.reshape(128, 16 * KV)),
            Whh2T=_bf(Whh2T),
            EmbOutT=_bf(EmbOutT.reshape(128, 2 * V)),
            onehotT=_bf(oh),
            ctxT0=_bf(ctxT0),
        ))
    return per_core, ids, enc


INPUT_SPECS = [
    ("keyT", [128, BL * T], BF16),
    ("v_sb", [128, NCH * BL * KV], BF16),
    ("tailT", [48, 1], F32),
    ("Wr1T", [128, 16 * H], BF16),
    ("Wc1T", [128, 4 * H], BF16),
    ("EW", [V, 4 * H], BF16),
    ("Wih2T", [128, 16 * KV], BF16),
    ("Whh2T", [128, 4 * KV], BF16),
    ("EmbOutT", [128, 2 * V], BF16),
    ("onehotT", [V, L * BL], BF16),
    ("ctxT0", [KV, BL], BF16),
]


def build_program(S=L, split_waits=True):
    nc = bass.Bass("TRN2", target_bir_lowering=False, debug=False,
                   num_devices=1)
    ins = {}
    for name, shape, dt in INPUT_SPECS:
        ins[name] = nc.dram_tensor(name, shape, dt, kind="ExternalInput").ap()
    preds_d = nc.dram_tensor("preds", [BL, S * V], F32,
                             kind="ExternalOutput").ap()
    attn_d = nc.dram_tensor("attn_raw", [S, T], BF16,
                            kind="ExternalOutput").ap()
    sums_d = nc.dram_tensor("sums0", [1, S], F32, kind="ExternalOutput").ap()

    with tile.TileContext(nc) as tc:
        with ExitStack() as ctx:
            build_body(ctx, tc, ins, preds_d, attn_d, sums_d, S)
    if split_waits:   # needed for walrus; CoreSim rejects the bare NOPs
        split_multi_waits(nc)
    return nc


def build_body(ctx, tc, ins, preds_d, attn_d, sums_d, S):
    nc = tc.nc
    const = ctx.enter_context(tc.tile_pool(name="const", bufs=1))
    state = ctx.enter_context(tc.tile_pool(name="state", bufs=2))
    work = ctx.enter_context(tc.tile_pool(name="work", bufs=2))
    ps_const = ctx.enter_context(
        tc.tile_pool(name="ps_const", bufs=1, space="PSUM"))

    # ---- load constants into SBUF
    sb = {}
    for name, shape, dt in INPUT_SPECS:
        t = const.tile(shape, dt, tag=f"in_{name}")
        nc.sync.dma_start(t[:], ins[name][:])
        sb[name] = t

    logits_buf = const.tile([BL, S * V], F32, tag="logits_buf")
    sums_buf = const.tile([1, S], F32, tag="sums_buf")

    # ---- state: h/ctx bf16 (matmul stationaries), cd = 2*c fp32
    h1T = const.tile([128, 4 * BL], BF16, tag="h1T_init")
    h2T = const.tile([128, BL], BF16, tag="h2T_init")
    c1T = const.tile([128, 4 * BL], F32, tag="c1T_init")
    c2T = const.tile([128, BL], F32, tag="c2T_init")
    ctxT_full = const.tile([KV, GP], BF16, tag="ctxT_init")
    # masked query stationaries: block b = [128, 4] with only col b%4 = q_b;
    # energy group j (batches 4j..4j+3) lands at psum rows 32j..32j+4, so
    # per-batch recip slices stay 32-aligned for the fused ctx normalize
    qmask = const.tile([128, BL * 4], BF16, tag="qmask")
    for t_ in (h1T, h2T, c1T, c2T, ctxT_full, qmask):
        nc.vector.memset(t_[:], 0.0)
    nc.vector.tensor_copy(ctxT_full[:, 0:BL], sb["ctxT0"][:])
    ctxT = ctxT_full[:, 0:BL]

    # once-allocated psum + padded staging tiles (memset so the garbage
    # regions are initialized; CoreSim rejects partially-uninit reads)
    g1ps = ps_const.tile([128, 512], F32, tag="g1ps")
    g2full = ps_const.tile([128, 512], F32, tag="g2ps")  # bank-width pitch
    g2ps = g2full[:, 0:128]
    ops = g2full[0:BL, 128:128 + V]
    eps = ps_const.tile([128, T], F32, tag="eps")
    cps = ps_const.tile([128, 512], F32, tag="cps")
    for t_ in (g1ps, g2full, eps, cps):
        nc.vector.memset(t_[:], 0.0)
    cd = const.tile([GP, 2 * 512], BF16, tag="cd")
    nc.vector.memset(cd[:], 0.0)

    Wr1T = sb["Wr1T"][:].rearrange("p (q n) -> p q n", q=4)
    Wih2T = sb["Wih2T"][:].rearrange("p (q n) -> p q n", q=4)
    EmbOutT = sb["EmbOutT"][:].rearrange("p (q n) -> p q n", q=2)
    v_r = v_sb_r = sb["v_sb"][:].rearrange("p (c b k) -> p c b k", c=NCH, b=BL)
    keyT_r = sb["keyT"][:].rearrange("p (b t) -> p b t", b=BL)

    def g1_onehot(i):
        """gates1 onehot chunk for step i — depends only on g1ps being
        free; emitted right after step i-1's psum copy to fill the PE gap
        during LSTM1 post-processing."""
        oh_i = sb["onehotT"][:, i * BL:(i + 1) * BL]
        for g in range(4):
            nc.tensor.matmul(g1ps[32 * g:32 * g + BL, :], oh_i,
                             sb["EW"][:, g * 512:(g + 1) * 512],
                             start=True, stop=False,
                             tile_position=(0, 32 * g),
                             skip_group_check=True)

    def g1_early(i):
        """gates1 h1 chunks for step i — depend only on h1T(i-1), so
        they're emitted during step i-1's attention to keep PE fed."""
        for g in range(4):
            out = g1ps[32 * g:32 * g + BL, :]
            tp = (0, 32 * g)
            for q in range(4):
                nc.tensor.matmul(out, h1T[:, q * BL:(q + 1) * BL],
                                 Wr1T[:, q, g * 512:(g + 1) * 512],
                                 start=False, stop=False, tile_position=tp,
                                 skip_group_check=True)

    def g1_late(i):
        """gates1 ctx chunk (accumulation-group tail)."""
        for g in range(4):
            nc.tensor.matmul(g1ps[32 * g:32 * g + BL, :], ctxT,
                             sb["Wc1T"][:, g * 512:(g + 1) * 512],
                             start=False, stop=True,
                             tile_position=(0, 32 * g),
                             skip_group_check=True)

    g1_onehot(0)
    g1_early(0)
    for i in range(S):
        # ================= LSTM1 gates =================
        g1_late(i)
        g1rows = work.tile([128, 512], BF16, tag="g1rows")
        nc.vector.tensor_copy(g1rows[:], g1ps[:])
        if i + 1 < S:
            g1_onehot(i + 1)
        g1T = work.tile([128, 4 * 128], BF16, tag="g1T")
        nc.sync.dma_start_transpose(
            g1T[:].rearrange("p (c q) -> p c q", c=4), g1rows[:])
        # th = tanh(0.5 * gates) for all gates (g-slice pre-doubled)
        th1 = work.tile([128, 4 * 128], F32, tag="th1")
        nc.scalar.activation(th1[:], g1T[:], AF.Tanh, scale=0.5)
        th1_r = th1[:].rearrange("p (c q) -> p c q", c=4)
        th_i = th1_r[:, :, 0:BL]
        th_f = th1_r[:, :, 32:32 + BL]
        th_g = th1_r[:, :, 64:64 + BL]
        th_o = th1_r[:, :, 96:96 + BL]
        # cd1 = (th_f+1)*cd1_old*0.5 + (th_i+1)*th_g   [cd = 2c]
        c1T_old = c1T
        c1T = state.tile([128, 4 * BL], F32, tag="c1T")
        c1_r = c1T[:].rearrange("p (q b) -> p q b", q=4)
        c1o_r = c1T_old[:].rearrange("p (q b) -> p q b", q=4)
        tmp1 = work.tile([128, 4 * BL], F32, tag="tmp1")
        tmp1_r = tmp1[:].rearrange("p (q b) -> p q b", q=4)
        tmp2 = work.tile([128, 4 * BL], F32, tag="tmp2")
        tmp2_r = tmp2[:].rearrange("p (q b) -> p q b", q=4)
        nc.vector.scalar_tensor_tensor(tmp1_r, th_f, 1.0, c1o_r,
                                       op0=OP.add, op1=OP.mult)
        nc.vector.scalar_tensor_tensor(tmp2_r, th_i, 1.0, th_g,
                                       op0=OP.add, op1=OP.mult)
        nc.vector.scalar_tensor_tensor(c1_r, tmp1_r, 0.5, tmp2_r,
                                       op0=OP.mult, op1=OP.add)
        # h1 = 0.5*(th_o+1)*tanh(c1);  tanh(c1) = tanh(0.5*cd1)
        tnc1 = work.tile([128, 4 * BL], F32, tag="tnc1")
        nc.scalar.activation(tnc1[:], c1T[:], AF.Tanh, scale=0.5)
        h1f = work.tile([128, 4 * BL], F32, tag="h1f")
        h1f_r = h1f[:].rearrange("p (q b) -> p q b", q=4)
        nc.vector.scalar_tensor_tensor(
            h1f_r, th_o, 1.0, tnc1[:].rearrange("p (q b) -> p q b", q=4),
            op0=OP.add, op1=OP.mult)
        h1T = state.tile([128, 4 * BL], BF16, tag="h1T")
        nc.vector.tensor_scalar_mul(h1T[:], h1f[:], 0.5)

        # ================= LSTM2 gates =================
        for g in range(4):
            out = g2ps[32 * g:32 * g + BL, :]
            tp = (0, 32 * g)
            for q in range(4):
                nc.tensor.matmul(out, h1T[:, q * BL:(q + 1) * BL],
                                 Wih2T[:, q, g * 128:(g + 1) * 128],
                                 start=(q == 0), stop=False, tile_position=tp)
            nc.tensor.matmul(out, h2T[:],
                             sb["Whh2T"][:, g * 128:(g + 1) * 128],
                             start=False, stop=True, tile_position=tp)
        g2rows = work.tile([128, 128], BF16, tag="g2rows")
        nc.vector.tensor_copy(g2rows[:], g2ps[:])
        g2T = work.tile([128, 128], BF16, tag="g2T")
        nc.sync.dma_start_transpose(
            g2T[:].rearrange("p (c q) -> p c q", c=1), g2rows[:])
        th2 = work.tile([128, 128], F32, tag="th2")
        nc.scalar.activation(th2[:], g2T[:], AF.Tanh, scale=0.5)
        t2_i = th2[:, 0:BL]; t2_f = th2[:, 32:32 + BL]
        t2_g = th2[:, 64:64 + BL]; t2_o = th2[:, 96:96 + BL]
        c2T_old = c2T
        c2T = state.tile([128, BL], F32, tag="c2T")
        tmp3 = work.tile([128, BL], F32, tag="tmp3")
        tmp4 = work.tile([128, BL], F32, tag="tmp4")
        nc.vector.scalar_tensor_tensor(tmp3[:], t2_f, 1.0, c2T_old[:],
                                       op0=OP.add, op1=OP.mult)
        nc.vector.scalar_tensor_tensor(tmp4[:], t2_i, 1.0, t2_g,
                                       op0=OP.add, op1=OP.mult)
        nc.vector.scalar_tensor_tensor(c2T[:], tmp3[:], 0.5, tmp4[:],
                                       op0=OP.mult, op1=OP.add)
        tnc2 = work.tile([128, BL], F32, tag="tnc2")
        nc.scalar.activation(tnc2[:], c2T[:], AF.Tanh, scale=0.5)
        h2f = work.tile([128, BL], F32, tag="h2f")
        nc.vector.scalar_tensor_tensor(h2f[:], t2_o, 1.0, tnc2[:],
                                       op0=OP.add, op1=OP.mult)
        h2T = state.tile([128, BL], BF16, tag="h2T")
        nc.vector.tensor_scalar_mul(h2T[:], h2f[:], 0.5)
        # scatter queries into masked stationaries (col b%4 of block b)
        nc.vector.tensor_copy(qmask[:, 0:20:5], h2T[:, 0:4])
        nc.vector.tensor_copy(qmask[:, 16:32:5], h2T[:, 4:8])
        # output projection, q chunk (ctx chunk joins after attention)
        nc.tensor.matmul(ops[:], h2T[:], EmbOutT[:, 0, :], start=True,
                         stop=False)

        # ================= attention: energy =================
        # 2 col groups; batch 4j+r's energy lands at psum row 32j+r
        for j in range(2):
            tp = (0, 32 * j)
            for r in range(4):
                b = 4 * j + r
                st = qmask[:, b * 4:(b + 1) * 4]
                for ns in range(4):
                    nc.tensor.matmul(
                        eps[32 * j:32 * j + 4, ns * 512:(ns + 1) * 512],
                        st, keyT_r[:, b, ns * 512:(ns + 1) * 512],
                        start=(r == 0), stop=(r == 3), tile_position=tp)
        expE = work.tile([48, T], BF16, tag="expE")
        sumsP = work.tile([48, 1], F32, tag="sumsP")
        nc.scalar.activation(expE[:], eps[0:48, :], AF.Exp, scale=SCALE,
                             accum_out=sumsP[:])
        # valid-only denominator: full sum minus (T - len_b) of exp(0)=1
        sums_cor = work.tile([48, 1], F32, tag="sums_cor")
        nc.vector.scalar_tensor_tensor(sums_cor[:], sb["tailT"][:], -1.0,
                                       sumsP[:], op0=OP.mult, op1=OP.add)
        recipP = work.tile([48, 1], F32, tag="recipP")
        nc.vector.reciprocal(recipP[:], sums_cor[:])
        nc.vector.tensor_copy(sums_buf[:, i:i + 1], sums_cor[0:1, :])
        # attn_plot raw row (host divides by sums0 and zeroes the tail)
        nc.gpsimd.dma_start(attn_d[i:i + 1, :], expE[0:1, :])
        # fill the PE gap while exp/transposes run elsewhere
        if i + 1 < S:
            g1_early(i + 1)
        expT_A = work.tile([128, NCH * GP], BF16, tag="expT_A")
        nc.sync.dma_start_transpose(
            expT_A[:].rearrange("p (c b) -> p c b", c=NCH), expE[0:16, :])
        expT_B = work.tile([128, NCH * GP], BF16, tag="expT_B")
        nc.sync.dma_start_transpose(
            expT_B[:].rearrange("p (c b) -> p c b", c=NCH), expE[32:48, :])
        eT = [expT_A[:].rearrange("p (c b) -> p c b", c=NCH),
              expT_B[:].rearrange("p (c b) -> p c b", c=NCH)]

        # ================= attention: context =================
        # 4-batch diagonal: group j (batches 4j..4j+3), M=4 stationary
        # (unnormalized exp weights), valid ctx_b (b=4j+r) at psum
        # (row 32j+r, cols 128r:128r+128); garbage elsewhere unread.
        for j in range(2):
            tp = (0, 32 * j)
            out = cps[32 * j:32 * j + 4, :]
            for c in range(NCH):
                nc.tensor.matmul(
                    out, eT[j][:, c, 0:4],
                    v_r[:, c, 4 * j:4 * j + 4, :].rearrange(
                        "p b k -> p (b k)"),
                    start=(c == 0), stop=(c == NCH - 1), tile_position=tp)
        # extract + normalize fused: per-partition recip slices are
        # 32-aligned thanks to the row-{0..3,32..35} energy layout
        nc.vector.tensor_scalar_mul(cd[0:4, 0:512], cps[0:4, :],
                                    recipP[0:4, :])
        nc.vector.tensor_scalar_mul(cd[0:4, 512:1024], cps[32:36, :],
                                    recipP[32:36, :])
        ctxT3 = state.tile([KV, BL * GP], BF16, tag="ctxT")
        nc.sync.dma_start_transpose(
            ctxT3[:].rearrange("p (c b) -> p c b", c=BL), cd[:])
        ctxT_m = state.tile([KV, BL], BF16, tag="ctxT_m")
        nc.vector.tensor_copy(
            ctxT_m[:],
            ctxT3[:].rearrange("p (j x) -> p j x", j=2)[:, :, 0:52:17])
        ctxT = ctxT_m[:]

        # ================= output projection (ctx chunk) =================
        nc.tensor.matmul(ops[:], ctxT, EmbOutT[:, 1, :], start=False,
                         stop=True)
        nc.vector.tensor_copy(logits_buf[:, i * V:(i + 1) * V], ops[:])

    nc.sync.dma_start(preds_d[:], logits_buf[:])
    nc.sync.dma_start(sums_d[:], sums_buf[:])


# ---------------------------------------------------------------------------
def run_compiled(nc, per_core):
    from concourse.bass_utils import run_bass_kernel_spmd
    in_maps = [per_core[c] for c in range(NCORES)]
    return run_bass_kernel_spmd(nc, in_maps, core_ids=list(range(NCORES)))


def assemble_outputs(results, enc, S=L):
    preds = np.zeros((B, S, V), np.float32)
    for c in range(NCORES):
        preds[c * BL:(c + 1) * BL] = \
            np.asarray(results[c]["preds"]).reshape(BL, S, V)
    attn_plot = np.asarray(results[0]["attn_raw"]).astype(np.float32)
    sums0 = np.asarray(results[0]["sums0"]).reshape(-1)
    attn_plot = attn_plot / sums0[:, None]
    attn_plot[:, int(enc[0]):] = 0.0   # masked cols carry exp(0)/sum
    return preds, attn_plot


def kernel(key, value, encoder_len, y, emb_weight,
           W_ih1, W_hh1, b_ih1, b_hh1, W_ih2, W_hh2, b_ih2, b_hh2,
           out_bias):
    """Full-input entry point. Biases are structurally zero in this problem
    (setup_inputs zero-fills them); asserted below."""
    for bias in (b_ih1, b_hh1, b_ih2, b_hh2, out_bias):
        assert np.abs(np.asarray(bias)).max() == 0.0, "nonzero bias unsupported"

    per_core, ids, enc = host_prep(key, value, encoder_len, y, emb_weight,
                                   W_ih1, W_hh1, W_ih2, W_hh2)
    nc = build_program(L)
    res = run_compiled(nc, per_core)
    return assemble_outputs(res.results, enc, L)


# revision 62
# speedup vs baseline: 1.4766x; 1.4766x over previous
"""Trainium2 Bass kernel for nn_Decoder (LAS-style attention decoder).

Data-parallel over batch: 64 batches -> 8 cores x 8 batches. Per core, the
full 256-step recurrence runs on-chip: key/value/weights live in SBUF (bf16),
LSTM gates / attention are PE matmuls with small stationaries and streamed
rhs, softmax exp on ACT, elementwise on DVE, layout flips via DMA transpose.

Host (numpy) does all data layout: transposed/prescaled weight tables,
one-hot id table, zero-padded key/value tails (mask), value ones-column
(softmax denominator), initial context. The device program itself is
input-independent.

Math notes:
 - sigmoid(x) = 0.5*tanh(0.5x)+0.5; the cell-input gate g needs tanh(g), so
   its weight columns are pre-scaled by 2 on host and ONE ACT tanh with
   scale=0.5 covers all four gates.
 - cell state c is stored doubled (cd = 2c) so the 0.5 factors fold into
   single scalar_tensor_tensor ops and the ACT tanh scale.
 - softmax: energies are bounded (|q|<=1), so exp without max-subtraction is
   safe; masked t positions have key/value zeroed on host, the value ones-
   column accumulates the valid-only denominator during the context matmul.
"""
import os
import sys
import math
from contextlib import ExitStack

import numpy as np
import ml_dtypes

for _p in ("/root/.axon_site", "/root/.axon_site/_ro/trn_rl_repo",
           "/root/.axon_site/_ro/pypackages"):
    if _p not in sys.path and os.path.isdir(_p):
        sys.path.append(_p)

import concourse.bass as bass
import concourse.tile as tile
from concourse import mybir
from concourse._compat import not_none as nn

F32 = mybir.dt.float32
BF16 = mybir.dt.bfloat16
AF = mybir.ActivationFunctionType
OP = mybir.AluOpType

# ---------------------------------------------------------------------------
# Workaround: this walrus build rejects multi-wait Drain instructions
# (setupSyncWait TPB_CTRL "Too many sync wait commands"). Split the Tile
# kernel-tail drain into single-wait drains.
_orig_drain_and_barrier = tile.TileContext._drain_and_barrier

def _patched_drain_and_barrier(self, tick_clock, wait_clock):
    from concourse.tile import ScopedClock
    drain_inst = self.nc.sync.drain()
    wait_clock.add_sem_waits(drain_inst.ins,
                             ScopedClock({None: tick_clock.global_clock}))
    si = drain_inst.ins.sync_info
    waits = list(si.on_wait or []) if si is not None else []
    if len(waits) > 1:
        bb = nn(self.nc.cur_bb).bb
        drain_inst.ins.sync_info.on_wait = waits[:1]
        idx = bb.instructions.index(drain_inst.ins)
        for w in waits[1:]:
            extra = self.nc.sync.drain()
            extra.ins.sync_info = mybir.SyncInfo(on_wait=[w], on_update=[])
            bb.instructions.remove(extra.ins)
            bb.instructions.insert(idx, extra.ins)
            idx += 1
    self.nc.all_engine_barrier()
    assert self.sems is not None
    popped = self.nc._tile_sem_poison_stack.pop()
    assert popped is self._sem_poison
    self.nc.clear_and_free_semaphores(list(self.sems.allocated().values()))
    self.nc.all_engine_barrier()

tile.TileContext._drain_and_barrier = _patched_drain_and_barrier


def split_multi_waits(nc):
    """This walrus build allows only one sync-wait per instruction in several
    ISA structs. Hoist all-but-one wait onto same-engine NOPs placed before
    the instruction (the engine queue blocks on them in order)."""
    n = 0
    for f in nc.m.functions:
        for bb in f.blocks:
            out = []
            for ins in bb.instructions:
                si = getattr(ins, "sync_info", None)
                waits = list(si.on_wait) if si is not None and si.on_wait else []
                if len(waits) > 1:
                    for w in waits[:-1]:
                        n += 1
                        nop = mybir.InstNoOp(
                            name=nc.get_next_instruction_name(),
                            engine=ins.engine, ins=[], outs=[],
                            sync_info=mybir.SyncInfo(on_wait=[w],
                                                     on_update=[]))
                        out.append(nop)
                    ins.sync_info.on_wait = waits[-1:]
                out.append(ins)
            bb.instructions[:] = out
    return n

# ---------------------------------------------------------------------------
# Problem constants (hardcoded per spec)
B, T, KV, E, H, V, L = 64, 2048, 128, 256, 512, 30, 256
NCORES = 8
BL = B // NCORES          # 8 batches per core
NCH = T // 128            # 16 t-chunks
SOS, EOS = 1, 0
SCALE = 1.0 / math.sqrt(KV)
GP = 16                   # padded gather partitions (dma transpose needs %16)


def _bf(x):
    return np.ascontiguousarray(np.asarray(x, np.float32).astype(ml_dtypes.bfloat16))


def _f32(x):
    return np.ascontiguousarray(np.asarray(x, np.float32))


def host_prep(key, value, encoder_len, y, emb_weight, W_ih1, W_hh1,
              W_ih2, W_hh2):
    """Build per-core device input tensors (all layout work on host)."""
    key = _f32(key).copy()
    value = _f32(value).copy()
    enc = np.asarray(encoder_len).astype(np.int64)
    y = np.asarray(y)
    emb_weight = _f32(emb_weight)
    W_ih1 = _f32(W_ih1); W_hh1 = _f32(W_hh1)
    W_ih2 = _f32(W_ih2); W_hh2 = _f32(W_hh2)

    # initial context: mean over FULL T of raw value (reference semantics)
    ctx0 = value.mean(axis=1)                      # (B, KV) fp32

    # gate g-slice prescale by 2 (tanh(g) = tanh(2g * 0.5))
    def g2(W, D):
        W = W.copy(); W[2 * D:3 * D, :] *= 2.0; return W

    EW = (emb_weight @ W_ih1[:, :E].T)             # (V, 4H)
    EW = g2(EW.T, H).T
    Wc1 = g2(W_ih1[:, E:], H)                      # (4H, KV)
    W_hh1p = g2(W_hh1, H)
    W_ih2p = g2(W_ih2, KV)
    W_hh2p = g2(W_hh2, KV)

    # weight streams, rhs layout [K-chunk partitions, free]
    Wr1T = np.transpose(W_hh1p.T.reshape(4, 128, 4 * H), (1, 0, 2))
    Wc1T = Wc1.T                                   # (128, 2048)
    Wih2T = np.transpose(W_ih2p.T.reshape(4, 128, 4 * KV), (1, 0, 2))
    Whh2T = W_hh2p.T                               # (128, 512)
    EmbOutT = np.transpose(emb_weight.T.reshape(2, 128, V), (1, 0, 2))

    ids = np.concatenate([np.full((B, 1), SOS, y.dtype), y[:, :-1]], axis=1)

    # zero mask tails (after ctx0 which uses raw value)
    for b in range(B):
        key[b, enc[b]:, :] = 0.0
        value[b, enc[b]:, :] = 0.0

    per_core = []
    for c in range(NCORES):
        sl = slice(c * BL, (c + 1) * BL)
        k_c = key[sl]
        v_c = value[sl]
        keyT = np.transpose(k_c, (2, 0, 1))        # (128, BL, T)
        v4 = v_c.reshape(BL, NCH, 128, KV)         # (b, c, p, kv)
        v_sb = np.transpose(v4, (2, 1, 0, 3))      # (p, c, b, kv)
        # softmax denominator correction: exp(0)=1 at masked t, so
        # valid_sum = full_sum - (T - len_b); batch 4j+r at energy row 32j+r
        tailT = np.zeros((48, 1), np.float32)
        for b in range(BL):
            tailT[32 * (b // 4) + (b % 4), 0] = float(T - int(enc[c * BL + b]))
        oh = np.zeros((V, L * BL), np.float32)
        for b in range(BL):
            oh[ids[c * BL + b], np.arange(L) * BL + b] = 1.0
        ctxT0 = ctx0[sl].T                         # (KV, BL)
        per_core.append(dict(
            keyT=_bf(keyT.reshape(128, BL * T)),
            v_sb=_bf(v_sb.reshape(128, NCH * BL * KV)),
            tailT=tailT,
            Wr1T=_bf(Wr1T.reshape(128, 16 * H)),
            Wc1T=_bf(Wc1T),
            EW=_bf(EW),
            Wih2T=_bf(Wih2T.reshape(128, 16 * KV)),
            Whh2T=_bf(Whh2T),
            EmbOutT=_bf(EmbOutT.reshape(128, 2 * V)),
            onehotT=_bf(oh),
            ctxT0=_bf(ctxT0),
        ))
    return per_core, ids, enc


INPUT_SPECS = [
    ("keyT", [128, BL * T], BF16),
    ("v_sb", [128, NCH * BL * KV], BF16),
    ("tailT", [48, 1], F32),
    ("Wr1T", [128, 16 * H], BF16),
    ("Wc1T", [128, 4 * H], BF16),
    ("EW", [V, 4 * H], BF16),
    ("Wih2T", [128, 16 * KV], BF16),
    ("Whh2T", [128, 4 * KV], BF16),
    ("EmbOutT", [128, 2 * V], BF16),
    ("onehotT", [V, L * BL], BF16),
    ("ctxT0", [KV, BL], BF16),
]


def build_program(S=L, split_waits=True):
    nc = bass.Bass("TRN2", target_bir_lowering=False, debug=False,
                   num_devices=1)
    ins = {}
    for name, shape, dt in INPUT_SPECS:
        ins[name] = nc.dram_tensor(name, shape, dt, kind="ExternalInput").ap()
    preds_d = nc.dram_tensor("preds", [BL, S * V], F32,
                             kind="ExternalOutput").ap()
    attn_d = nc.dram_tensor("attn_raw", [S, T], BF16,
                            kind="ExternalOutput").ap()
    sums_d = nc.dram_tensor("sums0", [1, S], F32, kind="ExternalOutput").ap()

    with tile.TileContext(nc) as tc:
        with ExitStack() as ctx:
            build_body(ctx, tc, ins, preds_d, attn_d, sums_d, S)
    if split_waits:   # needed for walrus; CoreSim rejects the bare NOPs
        split_multi_waits(nc)
    return nc


def build_body(ctx, tc, ins, preds_d, attn_d, sums_d, S):
    nc = tc.nc
    const = ctx.enter_context(tc.tile_pool(name="const", bufs=1))
    state = ctx.enter_context(tc.tile_pool(name="state", bufs=2))
    work = ctx.enter_context(tc.tile_pool(name="work", bufs=2))
    ps_const = ctx.enter_context(
        tc.tile_pool(name="ps_const", bufs=1, space="PSUM"))

    # ---- load constants into SBUF
    sb = {}
    for name, shape, dt in INPUT_SPECS:
        t = const.tile(shape, dt, tag=f"in_{name}")
        nc.sync.dma_start(t[:], ins[name][:])
        sb[name] = t

    logits_buf = const.tile([BL, S * V], F32, tag="logits_buf")
    sums_buf = const.tile([1, S], F32, tag="sums_buf")

    # ---- state: h/ctx bf16 (matmul stationaries), cd = 2*c fp32
    h1T = const.tile([128, 4 * BL], BF16, tag="h1T_init")
    h2T = const.tile([128, BL], BF16, tag="h2T_init")
    c1T = const.tile([128, 4 * BL], F32, tag="c1T_init")
    c2T = const.tile([128, BL], F32, tag="c2T_init")
    ctxT_full = const.tile([KV, GP], BF16, tag="ctxT_init")
    # masked query stationaries: block b = [128, 4] with only col b%4 = q_b;
    # energy group j (batches 4j..4j+3) lands at psum rows 32j..32j+4, so
    # per-batch recip slices stay 32-aligned for the fused ctx normalize
    qmask = const.tile([128, BL * 4], BF16, tag="qmask")
    for t_ in (h1T, h2T, c1T, c2T, ctxT_full, qmask):
        nc.vector.memset(t_[:], 0.0)
    nc.vector.tensor_copy(ctxT_full[:, 0:BL], sb["ctxT0"][:])
    ctxT = ctxT_full[:, 0:BL]

    # once-allocated psum + padded staging tiles (memset so the garbage
    # regions are initialized; CoreSim rejects partially-uninit reads)
    g1ps = ps_const.tile([128, 512], F32, tag="g1ps")
    g2full = ps_const.tile([128, 512], F32, tag="g2ps")  # bank-width pitch
    g2ps = g2full[:, 0:128]
    ops = g2full[0:BL, 128:128 + V]
    epsH = [ps_const.tile([128, T // 2], F32, tag="epsA", name="epsA"),
            ps_const.tile([128, T // 2], F32, tag="epsB", name="epsB")]
    cps = ps_const.tile([128, 512], F32, tag="cps")
    for t_ in (g1ps, g2full, epsH[0], epsH[1], cps):
        nc.vector.memset(t_[:], 0.0)
    cd = const.tile([GP, 2 * 512], BF16, tag="cd")
    nc.vector.memset(cd[:], 0.0)

    Wr1T = sb["Wr1T"][:].rearrange("p (q n) -> p q n", q=4)
    Wih2T = sb["Wih2T"][:].rearrange("p (q n) -> p q n", q=4)
    EmbOutT = sb["EmbOutT"][:].rearrange("p (q n) -> p q n", q=2)
    v_r = v_sb_r = sb["v_sb"][:].rearrange("p (c b k) -> p c b k", c=NCH, b=BL)
    keyT_r = sb["keyT"][:].rearrange("p (b t) -> p b t", b=BL)

    def g1_early(i):
        """gates1 h1/onehot chunks for step i — depend only on h1T(i-1),
        so they're emitted during step i-1's attention to keep PE fed."""
        oh_i = sb["onehotT"][:, i * BL:(i + 1) * BL]
        for g in range(4):
            out = g1ps[32 * g:32 * g + BL, :]
            tp = (0, 32 * g)
            for q in range(4):
                nc.tensor.matmul(out, h1T[:, q * BL:(q + 1) * BL],
                                 Wr1T[:, q, g * 512:(g + 1) * 512],
                                 start=(q == 0), stop=False, tile_position=tp,
                                 skip_group_check=True)
            nc.tensor.matmul(out, oh_i,
                             sb["EW"][:, g * 512:(g + 1) * 512],
                             start=False, stop=False, tile_position=tp,
                             skip_group_check=True)

    def g1_late(i):
        """gates1 ctx chunk (accumulation-group tail)."""
        for g in range(4):
            nc.tensor.matmul(g1ps[32 * g:32 * g + BL, :], ctxT,
                             sb["Wc1T"][:, g * 512:(g + 1) * 512],
                             start=False, stop=True,
                             tile_position=(0, 32 * g),
                             skip_group_check=True)

    g1_early(0)
    for i in range(S):
        # ================= LSTM1 gates =================
        g1_late(i)
        g1rows = work.tile([128, 512], BF16, tag="g1rows")
        nc.vector.tensor_copy(g1rows[:], g1ps[:])
        g1T = work.tile([128, 4 * 128], BF16, tag="g1T")
        nc.sync.dma_start_transpose(
            g1T[:].rearrange("p (c q) -> p c q", c=4), g1rows[:])
        # th = tanh(0.5 * gates) for all gates (g-slice pre-doubled)
        th1 = work.tile([128, 4 * 128], F32, tag="th1")
        nc.scalar.activation(th1[:], g1T[:], AF.Tanh, scale=0.5)
        th1_r = th1[:].rearrange("p (c q) -> p c q", c=4)
        th_i = th1_r[:, :, 0:BL]
        th_f = th1_r[:, :, 32:32 + BL]
        th_g = th1_r[:, :, 64:64 + BL]
        th_o = th1_r[:, :, 96:96 + BL]
        # cd1 = (th_f+1)*cd1_old*0.5 + (th_i+1)*th_g   [cd = 2c]
        c1T_old = c1T
        c1T = state.tile([128, 4 * BL], F32, tag="c1T")
        c1_r = c1T[:].rearrange("p (q b) -> p q b", q=4)
        c1o_r = c1T_old[:].rearrange("p (q b) -> p q b", q=4)
        tmp1 = work.tile([128, 4 * BL], F32, tag="tmp1")
        tmp1_r = tmp1[:].rearrange("p (q b) -> p q b", q=4)
        tmp2 = work.tile([128, 4 * BL], F32, tag="tmp2")
        tmp2_r = tmp2[:].rearrange("p (q b) -> p q b", q=4)
        nc.vector.scalar_tensor_tensor(tmp1_r, th_f, 1.0, c1o_r,
                                       op0=OP.add, op1=OP.mult)
        nc.vector.scalar_tensor_tensor(tmp2_r, th_i, 1.0, th_g,
                                       op0=OP.add, op1=OP.mult)
        nc.vector.scalar_tensor_tensor(c1_r, tmp1_r, 0.5, tmp2_r,
                                       op0=OP.mult, op1=OP.add)
        # h1 = 0.5*(th_o+1)*tanh(c1);  tanh(c1) = tanh(0.5*cd1)
        tnc1 = work.tile([128, 4 * BL], F32, tag="tnc1")
        nc.scalar.activation(tnc1[:], c1T[:], AF.Tanh, scale=0.5)
        h1f = work.tile([128, 4 * BL], F32, tag="h1f")
        h1f_r = h1f[:].rearrange("p (q b) -> p q b", q=4)
        nc.vector.scalar_tensor_tensor(
            h1f_r, th_o, 1.0, tnc1[:].rearrange("p (q b) -> p q b", q=4),
            op0=OP.add, op1=OP.mult)
        h1T = state.tile([128, 4 * BL], BF16, tag="h1T")
        nc.vector.tensor_scalar_mul(h1T[:], h1f[:], 0.5)

        # ================= LSTM2 gates =================
        for g in range(4):
            out = g2ps[32 * g:32 * g + BL, :]
            tp = (0, 32 * g)
            for q in range(4):
                nc.tensor.matmul(out, h1T[:, q * BL:(q + 1) * BL],
                                 Wih2T[:, q, g * 128:(g + 1) * 128],
                                 start=(q == 0), stop=False, tile_position=tp)
            nc.tensor.matmul(out, h2T[:],
                             sb["Whh2T"][:, g * 128:(g + 1) * 128],
                             start=False, stop=True, tile_position=tp)
        g2rows = work.tile([128, 128], BF16, tag="g2rows")
        nc.vector.tensor_copy(g2rows[:], g2ps[:])
        g2T = work.tile([128, 128], BF16, tag="g2T")
        nc.sync.dma_start_transpose(
            g2T[:].rearrange("p (c q) -> p c q", c=1), g2rows[:])
        th2 = work.tile([128, 128], F32, tag="th2")
        nc.scalar.activation(th2[:], g2T[:], AF.Tanh, scale=0.5)
        t2_i = th2[:, 0:BL]; t2_f = th2[:, 32:32 + BL]
        t2_g = th2[:, 64:64 + BL]; t2_o = th2[:, 96:96 + BL]
        c2T_old = c2T
        c2T = state.tile([128, BL], F32, tag="c2T")
        tmp3 = work.tile([128, BL], F32, tag="tmp3")
        tmp4 = work.tile([128, BL], F32, tag="tmp4")
        nc.vector.scalar_tensor_tensor(tmp3[:], t2_f, 1.0, c2T_old[:],
                                       op0=OP.add, op1=OP.mult)
        nc.vector.scalar_tensor_tensor(tmp4[:], t2_i, 1.0, t2_g,
                                       op0=OP.add, op1=OP.mult)
        nc.vector.scalar_tensor_tensor(c2T[:], tmp3[:], 0.5, tmp4[:],
                                       op0=OP.mult, op1=OP.add)
        tnc2 = work.tile([128, BL], F32, tag="tnc2")
        nc.scalar.activation(tnc2[:], c2T[:], AF.Tanh, scale=0.5)
        h2f = work.tile([128, BL], F32, tag="h2f")
        nc.vector.scalar_tensor_tensor(h2f[:], t2_o, 1.0, tnc2[:],
                                       op0=OP.add, op1=OP.mult)
        h2T = state.tile([128, BL], BF16, tag="h2T")
        nc.vector.tensor_scalar_mul(h2T[:], h2f[:], 0.5)
        # scatter queries into masked stationaries (col b%4 of block b)
        nc.vector.tensor_copy(qmask[:, 0:20:5], h2T[:, 0:4])
        nc.vector.tensor_copy(qmask[:, 16:32:5], h2T[:, 4:8])
        # output projection, q chunk (ctx chunk joins after attention)
        nc.tensor.matmul(ops[:], h2T[:], EmbOutT[:, 0, :], start=True,
                         stop=False)

        # ================= attention: energy (T-halved pipeline) ========
        # 2 col groups; batch 4j+r's energy lands at psum row 32j+r.
        # T halves use separate psum tiles so exp/transpose of half h
        # overlaps the energy matmuls of half h+1.
        expE_h, sums_h, eT = [], [], {}
        for h in range(2):
            eph = epsH[h]
            for j in range(2):
                tp = (0, 32 * j)
                for r in range(4):
                    b = 4 * j + r
                    st = qmask[:, b * 4:(b + 1) * 4]
                    for ns in range(2):
                        nsg = 2 * h + ns
                        nc.tensor.matmul(
                            eph[32 * j:32 * j + 4,
                                ns * 512:(ns + 1) * 512],
                            st, keyT_r[:, b, nsg * 512:(nsg + 1) * 512],
                            start=(r == 0), stop=(r == 3), tile_position=tp)
            ex = work.tile([48, T // 2], BF16, tag=f"expE{h}", name=f"expE{h}")
            sm = work.tile([48, 1], F32, tag=f"sumsP{h}", name=f"sumsP{h}")
            nc.scalar.activation(ex[:], eph[0:48, :], AF.Exp, scale=SCALE,
                                 accum_out=sm[:])
            expE_h.append(ex); sums_h.append(sm)
            nc.gpsimd.dma_start(
                attn_d[i:i + 1, h * (T // 2):(h + 1) * (T // 2)], ex[0:1, :])
            for j in range(2):
                tt = work.tile([128, (NCH // 2) * GP], BF16,
                               tag=f"expT{h}{j}", name=f"expT{h}{j}")
                nc.sync.dma_start_transpose(
                    tt[:].rearrange("p (c b) -> p c b", c=NCH // 2),
                    ex[32 * j:32 * j + 16, :])
                eT[(h, j)] = tt[:].rearrange("p (c b) -> p c b", c=NCH // 2)
            if h == 0 and i + 1 < S:
                g1_early(i + 1)   # fill PE while half-0 exp/transposes run
        # valid-only denominator: full sum minus (T - len_b) of exp(0)=1
        sums_cor = work.tile([48, 1], F32, tag="sums_cor")
        nc.vector.scalar_tensor_tensor(sums_cor[:], sb["tailT"][:], -1.0,
                                       sums_h[0][:], op0=OP.mult, op1=OP.add)
        nc.vector.scalar_tensor_tensor(sums_cor[:], sums_h[1][:], 1.0,
                                       sums_cor[:], op0=OP.mult, op1=OP.add)
        recipP = work.tile([48, 1], F32, tag="recipP")
        nc.vector.reciprocal(recipP[:], sums_cor[:])
        nc.vector.tensor_copy(sums_buf[:, i:i + 1], sums_cor[0:1, :])

        # ================= attention: context =================
        # 4-batch diagonal: group j (batches 4j..4j+3), M=4 stationary
        # (unnormalized exp weights), valid ctx_b (b=4j+r) at psum
        # (row 32j+r, cols 128r:128r+128); garbage elsewhere unread.
        for h in range(2):
            for j in range(2):
                tp = (0, 32 * j)
                out = cps[32 * j:32 * j + 4, :]
                for c8 in range(NCH // 2):
                    c = h * (NCH // 2) + c8
                    nc.tensor.matmul(
                        out, eT[(h, j)][:, c8, 0:4],
                        v_r[:, c, 4 * j:4 * j + 4, :].rearrange(
                            "p b k -> p (b k)"),
                        start=(c == 0), stop=(c == NCH - 1),
                        tile_position=tp)
        # extract + normalize fused: per-partition recip slices are
        # 32-aligned thanks to the row-{0..3,32..35} energy layout
        nc.vector.tensor_scalar_mul(cd[0:4, 0:512], cps[0:4, :],
                                    recipP[0:4, :])
        nc.vector.tensor_scalar_mul(cd[0:4, 512:1024], cps[32:36, :],
                                    recipP[32:36, :])
        ctxT3 = state.tile([KV, BL * GP], BF16, tag="ctxT")
        nc.sync.dma_start_transpose(
            ctxT3[:].rearrange("p (c b) -> p c b", c=BL), cd[:])
        ctxT_m = state.tile([KV, BL], BF16, tag="ctxT_m")
        nc.vector.tensor_copy(
            ctxT_m[:],
            ctxT3[:].rearrange("p (j x) -> p j x", j=2)[:, :, 0:52:17])
        ctxT = ctxT_m[:]

        # ================= output projection (ctx chunk) =================
        nc.tensor.matmul(ops[:], ctxT, EmbOutT[:, 1, :], start=False,
                         stop=True)
        nc.vector.tensor_copy(logits_buf[:, i * V:(i + 1) * V], ops[:])

    nc.sync.dma_start(preds_d[:], logits_buf[:])
    nc.sync.dma_start(sums_d[:], sums_buf[:])


# ---------------------------------------------------------------------------
def run_compiled(nc, per_core):
    from concourse.bass_utils import run_bass_kernel_spmd
    in_maps = [per_core[c] for c in range(NCORES)]
    return run_bass_kernel_spmd(nc, in_maps, core_ids=list(range(NCORES)))


def assemble_outputs(results, enc, S=L):
    preds = np.zeros((B, S, V), np.float32)
    for c in range(NCORES):
        preds[c * BL:(c + 1) * BL] = \
            np.asarray(results[c]["preds"]).reshape(BL, S, V)
    attn_plot = np.asarray(results[0]["attn_raw"]).astype(np.float32)
    sums0 = np.asarray(results[0]["sums0"]).reshape(-1)
    attn_plot = attn_plot / sums0[:, None]
    attn_plot[:, int(enc[0]):] = 0.0   # masked cols carry exp(0)/sum
    return preds, attn_plot


def kernel(key, value, encoder_len, y, emb_weight,
           W_ih1, W_hh1, b_ih1, b_hh1, W_ih2, W_hh2, b_ih2, b_hh2,
           out_bias):
    """Full-input entry point. Biases are structurally zero in this problem
    (setup_inputs zero-fills them); asserted below."""
    for bias in (b_ih1, b_hh1, b_ih2, b_hh2, out_bias):
        assert np.abs(np.asarray(bias)).max() == 0.0, "nonzero bias unsupported"

    per_core, ids, enc = host_prep(key, value, encoder_len, y, emb_weight,
                                   W_ih1, W_hh1, W_ih2, W_hh2)
    nc = build_program(L)
    res = run_compiled(nc, per_core)
    return assemble_outputs(res.results, enc, L)


# revision 64
# speedup vs baseline: 1.5455x; 1.0466x over previous
"""Trainium2 Bass kernel for nn_Decoder (LAS-style attention decoder).

Data-parallel over batch: 64 batches -> 8 cores x 8 batches. Per core, the
full 256-step recurrence runs on-chip: key/value/weights live in SBUF (bf16),
LSTM gates / attention are PE matmuls with small stationaries and streamed
rhs, softmax exp on ACT, elementwise on DVE, layout flips via DMA transpose.

Host (numpy) does all data layout: transposed/prescaled weight tables,
one-hot id table, zero-padded key/value tails (mask), value ones-column
(softmax denominator), initial context. The device program itself is
input-independent.

Math notes:
 - sigmoid(x) = 0.5*tanh(0.5x)+0.5; the cell-input gate g needs tanh(g), so
   its weight columns are pre-scaled by 2 on host and ONE ACT tanh with
   scale=0.5 covers all four gates.
 - cell state c is stored doubled (cd = 2c) so the 0.5 factors fold into
   single scalar_tensor_tensor ops and the ACT tanh scale.
 - softmax: energies are bounded (|q|<=1), so exp without max-subtraction is
   safe; masked t positions have key/value zeroed on host, the value ones-
   column accumulates the valid-only denominator during the context matmul.
"""
import os
import sys
import math
from contextlib import ExitStack

import numpy as np
import ml_dtypes

for _p in ("/root/.axon_site", "/root/.axon_site/_ro/trn_rl_repo",
           "/root/.axon_site/_ro/pypackages"):
    if _p not in sys.path and os.path.isdir(_p):
        sys.path.append(_p)

import concourse.bass as bass
import concourse.tile as tile
from concourse import mybir
from concourse._compat import not_none as nn

F32 = mybir.dt.float32
BF16 = mybir.dt.bfloat16
AF = mybir.ActivationFunctionType
OP = mybir.AluOpType

# ---------------------------------------------------------------------------
# Workaround: this walrus build rejects multi-wait Drain instructions
# (setupSyncWait TPB_CTRL "Too many sync wait commands"). Split the Tile
# kernel-tail drain into single-wait drains.
_orig_drain_and_barrier = tile.TileContext._drain_and_barrier

def _patched_drain_and_barrier(self, tick_clock, wait_clock):
    from concourse.tile import ScopedClock
    drain_inst = self.nc.sync.drain()
    wait_clock.add_sem_waits(drain_inst.ins,
                             ScopedClock({None: tick_clock.global_clock}))
    si = drain_inst.ins.sync_info
    waits = list(si.on_wait or []) if si is not None else []
    if len(waits) > 1:
        bb = nn(self.nc.cur_bb).bb
        drain_inst.ins.sync_info.on_wait = waits[:1]
        idx = bb.instructions.index(drain_inst.ins)
        for w in waits[1:]:
            extra = self.nc.sync.drain()
            extra.ins.sync_info = mybir.SyncInfo(on_wait=[w], on_update=[])
            bb.instructions.remove(extra.ins)
            bb.instructions.insert(idx, extra.ins)
            idx += 1
    self.nc.all_engine_barrier()
    assert self.sems is not None
    popped = self.nc._tile_sem_poison_stack.pop()
    assert popped is self._sem_poison
    self.nc.clear_and_free_semaphores(list(self.sems.allocated().values()))
    self.nc.all_engine_barrier()

tile.TileContext._drain_and_barrier = _patched_drain_and_barrier


def split_multi_waits(nc):
    """This walrus build allows only one sync-wait per instruction in several
    ISA structs. Hoist all-but-one wait onto same-engine NOPs placed before
    the instruction (the engine queue blocks on them in order)."""
    n = 0
    for f in nc.m.functions:
        for bb in f.blocks:
            out = []
            for ins in bb.instructions:
                si = getattr(ins, "sync_info", None)
                waits = list(si.on_wait) if si is not None and si.on_wait else []
                if len(waits) > 1:
                    for w in waits[:-1]:
                        n += 1
                        nop = mybir.InstNoOp(
                            name=nc.get_next_instruction_name(),
                            engine=ins.engine, ins=[], outs=[],
                            sync_info=mybir.SyncInfo(on_wait=[w],
                                                     on_update=[]))
                        out.append(nop)
                    ins.sync_info.on_wait = waits[-1:]
                out.append(ins)
            bb.instructions[:] = out
    return n

# ---------------------------------------------------------------------------
# Problem constants (hardcoded per spec)
B, T, KV, E, H, V, L = 64, 2048, 128, 256, 512, 30, 256
NCORES = 8
BL = B // NCORES          # 8 batches per core
NCH = T // 128            # 16 t-chunks
SOS, EOS = 1, 0
SCALE = 1.0 / math.sqrt(KV)
GP = 16                   # padded gather partitions (dma transpose needs %16)


def _bf(x):
    return np.ascontiguousarray(np.asarray(x, np.float32).astype(ml_dtypes.bfloat16))


def _f32(x):
    return np.ascontiguousarray(np.asarray(x, np.float32))


def host_prep(key, value, encoder_len, y, emb_weight, W_ih1, W_hh1,
              W_ih2, W_hh2):
    """Build per-core device input tensors (all layout work on host)."""
    key = _f32(key).copy()
    value = _f32(value).copy()
    enc = np.asarray(encoder_len).astype(np.int64)
    y = np.asarray(y)
    emb_weight = _f32(emb_weight)
    W_ih1 = _f32(W_ih1); W_hh1 = _f32(W_hh1)
    W_ih2 = _f32(W_ih2); W_hh2 = _f32(W_hh2)

    # initial context: mean over FULL T of raw value (reference semantics)
    ctx0 = value.mean(axis=1)                      # (B, KV) fp32

    # gate g-slice prescale by 2 (tanh(g) = tanh(2g * 0.5))
    def g2(W, D):
        W = W.copy(); W[2 * D:3 * D, :] *= 2.0; return W

    EW = (emb_weight @ W_ih1[:, :E].T)             # (V, 4H)
    EW = g2(EW.T, H).T
    Wc1 = g2(W_ih1[:, E:], H)                      # (4H, KV)
    W_hh1p = g2(W_hh1, H)
    W_ih2p = g2(W_ih2, KV)
    W_hh2p = g2(W_hh2, KV)

    # weight streams, rhs layout [K-chunk partitions, free]
    Wr1T = np.transpose(W_hh1p.T.reshape(4, 128, 4 * H), (1, 0, 2))
    Wc1T = Wc1.T                                   # (128, 2048)
    Wih2T = np.transpose(W_ih2p.T.reshape(4, 128, 4 * KV), (1, 0, 2))
    Whh2T = W_hh2p.T                               # (128, 512)
    EmbOutT = np.transpose(emb_weight.T.reshape(2, 128, V), (1, 0, 2))

    ids = np.concatenate([np.full((B, 1), SOS, y.dtype), y[:, :-1]], axis=1)

    # zero mask tails (after ctx0 which uses raw value)
    for b in range(B):
        key[b, enc[b]:, :] = 0.0
        value[b, enc[b]:, :] = 0.0

    per_core = []
    for c in range(NCORES):
        sl = slice(c * BL, (c + 1) * BL)
        k_c = key[sl]
        v_c = value[sl]
        keyT = np.transpose(k_c, (2, 0, 1))        # (128, BL, T)
        v4 = v_c.reshape(BL, NCH, 128, KV)         # (b, c, p, kv)
        v_sb = np.transpose(v4, (2, 1, 0, 3))      # (p, c, b, kv)
        # softmax denominator correction: exp(0)=1 at masked t, so
        # valid_sum = full_sum - (T - len_b); batch 4j+r at energy row 32j+r
        tailT = np.zeros((48, 1), np.float32)
        for b in range(BL):
            tailT[32 * (b // 4) + (b % 4), 0] = float(T - int(enc[c * BL + b]))
        oh = np.zeros((V, L * BL), np.float32)
        for b in range(BL):
            oh[ids[c * BL + b], np.arange(L) * BL + b] = 1.0
        ctxT0 = ctx0[sl].T                         # (KV, BL)
        per_core.append(dict(
            keyT=_bf(keyT.reshape(128, BL * T)),
            v_sb=_bf(v_sb.reshape(128, NCH * BL * KV)),
            tailT=tailT,
            Wr1T=_bf(Wr1T.reshape(128, 16 * H)),
            Wc1T=_bf(Wc1T),
            EW=_bf(EW),
            Wih2T=_bf(Wih2T.reshape(128, 16 * KV)),
            Whh2T=_bf(Whh2T),
            EmbOutT=_bf(EmbOutT.reshape(128, 2 * V)),
            onehotT=_bf(oh),
            ctxT0=_bf(ctxT0),
        ))
    return per_core, ids, enc


INPUT_SPECS = [
    ("keyT", [128, BL * T], BF16),
    ("v_sb", [128, NCH * BL * KV], BF16),
    ("tailT", [48, 1], F32),
    ("Wr1T", [128, 16 * H], BF16),
    ("Wc1T", [128, 4 * H], BF16),
    ("EW", [V, 4 * H], BF16),
    ("Wih2T", [128, 16 * KV], BF16),
    ("Whh2T", [128, 4 * KV], BF16),
    ("EmbOutT", [128, 2 * V], BF16),
    ("onehotT", [V, L * BL], BF16),
    ("ctxT0", [KV, BL], BF16),
]


def build_program(S=L, split_waits=True):
    nc = bass.Bass("TRN2", target_bir_lowering=False, debug=False,
                   num_devices=1)
    ins = {}
    for name, shape, dt in INPUT_SPECS:
        ins[name] = nc.dram_tensor(name, shape, dt, kind="ExternalInput").ap()
    preds_d = nc.dram_tensor("preds", [BL, S * V], F32,
                             kind="ExternalOutput").ap()
    attn_d = nc.dram_tensor("attn_raw", [S, T], BF16,
                            kind="ExternalOutput").ap()
    sums_d = nc.dram_tensor("sums0", [1, S], F32, kind="ExternalOutput").ap()

    with tile.TileContext(nc) as tc:
        with ExitStack() as ctx:
            build_body(ctx, tc, ins, preds_d, attn_d, sums_d, S)
    if split_waits:   # needed for walrus; CoreSim rejects the bare NOPs
        split_multi_waits(nc)
    return nc


def build_body(ctx, tc, ins, preds_d, attn_d, sums_d, S):
    nc = tc.nc
    const = ctx.enter_context(tc.tile_pool(name="const", bufs=1))
    state = ctx.enter_context(tc.tile_pool(name="state", bufs=2))
    work = ctx.enter_context(tc.tile_pool(name="work", bufs=2))
    ps_const = ctx.enter_context(
        tc.tile_pool(name="ps_const", bufs=1, space="PSUM"))

    # ---- load constants into SBUF
    sb = {}
    for name, shape, dt in INPUT_SPECS:
        t = const.tile(shape, dt, tag=f"in_{name}")
        nc.sync.dma_start(t[:], ins[name][:])
        sb[name] = t

    logits_buf = const.tile([BL, S * V], F32, tag="logits_buf")
    sums_buf = const.tile([1, S], F32, tag="sums_buf")

    # ---- state: h/ctx bf16 (matmul stationaries), cd = 2*c fp32
    h1T = const.tile([128, 4 * BL], BF16, tag="h1T_init")
    h2T = const.tile([128, BL], BF16, tag="h2T_init")
    c1T = const.tile([128, 4 * BL], F32, tag="c1T_init")
    c2T = const.tile([128, BL], F32, tag="c2T_init")
    ctxT_full = const.tile([KV, GP], BF16, tag="ctxT_init")
    # masked query stationaries: block b = [128, 4] with only col b%4 = q_b;
    # energy group j (batches 4j..4j+3) lands at psum rows 32j..32j+4, so
    # per-batch recip slices stay 32-aligned for the fused ctx normalize
    qmask = const.tile([128, BL * 4], BF16, tag="qmask")
    for t_ in (h1T, h2T, c1T, c2T, ctxT_full, qmask):
        nc.vector.memset(t_[:], 0.0)
    nc.vector.tensor_copy(ctxT_full[:, 0:BL], sb["ctxT0"][:])
    ctxT = ctxT_full[:, 0:BL]

    # once-allocated psum + padded staging tiles (memset so the garbage
    # regions are initialized; CoreSim rejects partially-uninit reads)
    g1ps = ps_const.tile([128, 512], F32, tag="g1ps")
    g2full = ps_const.tile([128, 512], F32, tag="g2ps")  # bank-width pitch
    g2ps = g2full[:, 0:128]
    ops = g2full[0:BL, 128:128 + V]
    epsH = [ps_const.tile([128, T // 2], F32, tag="epsA", name="epsA"),
            ps_const.tile([128, T // 2], F32, tag="epsB", name="epsB")]
    cps = ps_const.tile([128, 512], F32, tag="cps")
    for t_ in (g1ps, g2full, epsH[0], epsH[1], cps):
        nc.vector.memset(t_[:], 0.0)
    cd = const.tile([GP, 2 * 512], BF16, tag="cd")
    nc.vector.memset(cd[:], 0.0)

    Wr1T = sb["Wr1T"][:].rearrange("p (q n) -> p q n", q=4)
    Wih2T = sb["Wih2T"][:].rearrange("p (q n) -> p q n", q=4)
    EmbOutT = sb["EmbOutT"][:].rearrange("p (q n) -> p q n", q=2)
    v_r = v_sb_r = sb["v_sb"][:].rearrange("p (c b k) -> p c b k", c=NCH, b=BL)
    keyT_r = sb["keyT"][:].rearrange("p (b t) -> p b t", b=BL)

    def g1_early(i):
        """gates1 h1/onehot chunks for step i — depend only on h1T(i-1),
        so they're emitted during step i-1's attention to keep PE fed."""
        oh_i = sb["onehotT"][:, i * BL:(i + 1) * BL]
        for g in range(4):
            out = g1ps[32 * g:32 * g + BL, :]
            tp = (0, 32 * g)
            for q in range(4):
                nc.tensor.matmul(out, h1T[:, q * BL:(q + 1) * BL],
                                 Wr1T[:, q, g * 512:(g + 1) * 512],
                                 start=(q == 0), stop=False, tile_position=tp,
                                 skip_group_check=True)
            nc.tensor.matmul(out, oh_i,
                             sb["EW"][:, g * 512:(g + 1) * 512],
                             start=False, stop=False, tile_position=tp,
                             skip_group_check=True)

    def g1_late(i):
        """gates1 ctx chunk (accumulation-group tail)."""
        for g in range(4):
            nc.tensor.matmul(g1ps[32 * g:32 * g + BL, :], ctxT,
                             sb["Wc1T"][:, g * 512:(g + 1) * 512],
                             start=False, stop=True,
                             tile_position=(0, 32 * g),
                             skip_group_check=True)

    g1_early(0)
    for i in range(S):
        # ================= LSTM1 gates =================
        g1_late(i)
        # tanh straight off PSUM (ACT reads psum, writes bf16), THEN the
        # layout flip: one engine hop shorter than copy->transpose->tanh
        g1rows = work.tile([128, 512], BF16, tag="g1rows")
        nc.scalar.activation(g1rows[:], g1ps[:], AF.Tanh, scale=0.5)
        th1 = work.tile([128, 4 * 128], BF16, tag="th1")
        nc.sync.dma_start_transpose(
            th1[:].rearrange("p (c q) -> p c q", c=4), g1rows[:])
        th1_r = th1[:].rearrange("p (c q) -> p c q", c=4)
        th_i = th1_r[:, :, 0:BL]
        th_f = th1_r[:, :, 32:32 + BL]
        th_g = th1_r[:, :, 64:64 + BL]
        th_o = th1_r[:, :, 96:96 + BL]
        # cd1 = (th_f+1)*cd1_old*0.5 + (th_i+1)*th_g   [cd = 2c]
        c1T_old = c1T
        c1T = state.tile([128, 4 * BL], F32, tag="c1T")
        c1_r = c1T[:].rearrange("p (q b) -> p q b", q=4)
        c1o_r = c1T_old[:].rearrange("p (q b) -> p q b", q=4)
        tmp1 = work.tile([128, 4 * BL], F32, tag="tmp1")
        tmp1_r = tmp1[:].rearrange("p (q b) -> p q b", q=4)
        tmp2 = work.tile([128, 4 * BL], F32, tag="tmp2")
        tmp2_r = tmp2[:].rearrange("p (q b) -> p q b", q=4)
        nc.vector.scalar_tensor_tensor(tmp1_r, th_f, 1.0, c1o_r,
                                       op0=OP.add, op1=OP.mult)
        nc.vector.scalar_tensor_tensor(tmp2_r, th_i, 1.0, th_g,
                                       op0=OP.add, op1=OP.mult)
        nc.vector.scalar_tensor_tensor(c1_r, tmp1_r, 0.5, tmp2_r,
                                       op0=OP.mult, op1=OP.add)
        # h1 = 0.5*(th_o+1)*tanh(c1);  tanh(c1) = tanh(0.5*cd1)
        tnc1 = work.tile([128, 4 * BL], F32, tag="tnc1")
        nc.scalar.activation(tnc1[:], c1T[:], AF.Tanh, scale=0.5)
        h1f = work.tile([128, 4 * BL], F32, tag="h1f")
        h1f_r = h1f[:].rearrange("p (q b) -> p q b", q=4)
        nc.vector.scalar_tensor_tensor(
            h1f_r, th_o, 1.0, tnc1[:].rearrange("p (q b) -> p q b", q=4),
            op0=OP.add, op1=OP.mult)
        h1T = state.tile([128, 4 * BL], BF16, tag="h1T")
        nc.vector.tensor_scalar_mul(h1T[:], h1f[:], 0.5)

        # ================= LSTM2 gates =================
        for g in range(4):
            out = g2ps[32 * g:32 * g + BL, :]
            tp = (0, 32 * g)
            for q in range(4):
                nc.tensor.matmul(out, h1T[:, q * BL:(q + 1) * BL],
                                 Wih2T[:, q, g * 128:(g + 1) * 128],
                                 start=(q == 0), stop=False, tile_position=tp)
            nc.tensor.matmul(out, h2T[:],
                             sb["Whh2T"][:, g * 128:(g + 1) * 128],
                             start=False, stop=True, tile_position=tp)
        g2rows = work.tile([128, 128], BF16, tag="g2rows")
        nc.scalar.activation(g2rows[:], g2ps[:], AF.Tanh, scale=0.5)
        th2 = work.tile([128, 128], BF16, tag="th2")
        nc.sync.dma_start_transpose(
            th2[:].rearrange("p (c q) -> p c q", c=1), g2rows[:])
        t2_i = th2[:, 0:BL]; t2_f = th2[:, 32:32 + BL]
        t2_g = th2[:, 64:64 + BL]; t2_o = th2[:, 96:96 + BL]
        c2T_old = c2T
        c2T = state.tile([128, BL], F32, tag="c2T")
        tmp3 = work.tile([128, BL], F32, tag="tmp3")
        tmp4 = work.tile([128, BL], F32, tag="tmp4")
        nc.vector.scalar_tensor_tensor(tmp3[:], t2_f, 1.0, c2T_old[:],
                                       op0=OP.add, op1=OP.mult)
        nc.vector.scalar_tensor_tensor(tmp4[:], t2_i, 1.0, t2_g,
                                       op0=OP.add, op1=OP.mult)
        nc.vector.scalar_tensor_tensor(c2T[:], tmp3[:], 0.5, tmp4[:],
                                       op0=OP.mult, op1=OP.add)
        tnc2 = work.tile([128, BL], F32, tag="tnc2")
        nc.scalar.activation(tnc2[:], c2T[:], AF.Tanh, scale=0.5)
        h2f = work.tile([128, BL], F32, tag="h2f")
        nc.vector.scalar_tensor_tensor(h2f[:], t2_o, 1.0, tnc2[:],
                                       op0=OP.add, op1=OP.mult)
        h2T = state.tile([128, BL], BF16, tag="h2T")
        nc.vector.tensor_scalar_mul(h2T[:], h2f[:], 0.5)
        # scatter queries into masked stationaries (col b%4 of block b)
        nc.vector.tensor_copy(qmask[:, 0:20:5], h2T[:, 0:4])
        nc.vector.tensor_copy(qmask[:, 16:32:5], h2T[:, 4:8])
        # output projection, q chunk (ctx chunk joins after attention)
        nc.tensor.matmul(ops[:], h2T[:], EmbOutT[:, 0, :], start=True,
                         stop=False)

        # ================= attention: energy (T-halved pipeline) ========
        # 2 col groups; batch 4j+r's energy lands at psum row 32j+r.
        # T halves use separate psum tiles so exp/transpose of half h
        # overlaps the energy matmuls of half h+1.
        expE_h, sums_h, eT = [], [], {}
        for h in range(2):
            eph = epsH[h]
            for j in range(2):
                tp = (0, 32 * j)
                for r in range(4):
                    b = 4 * j + r
                    st = qmask[:, b * 4:(b + 1) * 4]
                    for ns in range(2):
                        nsg = 2 * h + ns
                        nc.tensor.matmul(
                            eph[32 * j:32 * j + 4,
                                ns * 512:(ns + 1) * 512],
                            st, keyT_r[:, b, nsg * 512:(nsg + 1) * 512],
                            start=(r == 0), stop=(r == 3), tile_position=tp)
            ex = work.tile([48, T // 2], BF16, tag=f"expE{h}", name=f"expE{h}")
            sm = work.tile([48, 1], F32, tag=f"sumsP{h}", name=f"sumsP{h}")
            nc.scalar.activation(ex[:], eph[0:48, :], AF.Exp, scale=SCALE,
                                 accum_out=sm[:])
            expE_h.append(ex); sums_h.append(sm)
            nc.gpsimd.dma_start(
                attn_d[i:i + 1, h * (T // 2):(h + 1) * (T // 2)], ex[0:1, :])
            for j in range(2):
                tt = work.tile([128, (NCH // 2) * GP], BF16,
                               tag=f"expT{h}{j}", name=f"expT{h}{j}")
                nc.sync.dma_start_transpose(
                    tt[:].rearrange("p (c b) -> p c b", c=NCH // 2),
                    ex[32 * j:32 * j + 16, :])
                eT[(h, j)] = tt[:].rearrange("p (c b) -> p c b", c=NCH // 2)
            if h == 0 and i + 1 < S:
                g1_early(i + 1)   # fill PE while half-0 exp/transposes run
        # valid-only denominator: full sum minus (T - len_b) of exp(0)=1
        sums_cor = work.tile([48, 1], F32, tag="sums_cor")
        nc.vector.scalar_tensor_tensor(sums_cor[:], sb["tailT"][:], -1.0,
                                       sums_h[0][:], op0=OP.mult, op1=OP.add)
        nc.vector.scalar_tensor_tensor(sums_cor[:], sums_h[1][:], 1.0,
                                       sums_cor[:], op0=OP.mult, op1=OP.add)
        recipP = work.tile([48, 1], F32, tag="recipP")
        nc.vector.reciprocal(recipP[:], sums_cor[:])
        nc.vector.tensor_copy(sums_buf[:, i:i + 1], sums_cor[0:1, :])

        # ================= attention: context =================
        # 4-batch diagonal: group j (batches 4j..4j+3), M=4 stationary
        # (unnormalized exp weights), valid ctx_b (b=4j+r) at psum
        # (row 32j+r, cols 128r:128r+128); garbage elsewhere unread.
        for h in range(2):
            for j in range(2):
                tp = (0, 32 * j)
                out = cps[32 * j:32 * j + 4, :]
                for c8 in range(NCH // 2):
                    c = h * (NCH // 2) + c8
                    nc.tensor.matmul(
                        out, eT[(h, j)][:, c8, 0:4],
                        v_r[:, c, 4 * j:4 * j + 4, :].rearrange(
                            "p b k -> p (b k)"),
                        start=(c == 0), stop=(c == NCH - 1),
                        tile_position=tp)
        # extract + normalize fused: per-partition recip slices are
        # 32-aligned thanks to the row-{0..3,32..35} energy layout
        nc.vector.tensor_scalar_mul(cd[0:4, 0:512], cps[0:4, :],
                                    recipP[0:4, :])
        nc.vector.tensor_scalar_mul(cd[0:4, 512:1024], cps[32:36, :],
                                    recipP[32:36, :])
        ctxT3 = state.tile([KV, BL * GP], BF16, tag="ctxT")
        nc.sync.dma_start_transpose(
            ctxT3[:].rearrange("p (c b) -> p c b", c=BL), cd[:])
        ctxT_m = state.tile([KV, BL], BF16, tag="ctxT_m")
        nc.vector.tensor_copy(
            ctxT_m[:],
            ctxT3[:].rearrange("p (j x) -> p j x", j=2)[:, :, 0:52:17])
        ctxT = ctxT_m[:]

        # ================= output projection (ctx chunk) =================
        nc.tensor.matmul(ops[:], ctxT, EmbOutT[:, 1, :], start=False,
                         stop=True)
        nc.vector.tensor_copy(logits_buf[:, i * V:(i + 1) * V], ops[:])

    nc.sync.dma_start(preds_d[:], logits_buf[:])
    nc.sync.dma_start(sums_d[:], sums_buf[:])


# ---------------------------------------------------------------------------
def run_compiled(nc, per_core):
    from concourse.bass_utils import run_bass_kernel_spmd
    in_maps = [per_core[c] for c in range(NCORES)]
    return run_bass_kernel_spmd(nc, in_maps, core_ids=list(range(NCORES)))


def assemble_outputs(results, enc, S=L):
    preds = np.zeros((B, S, V), np.float32)
    for c in range(NCORES):
        preds[c * BL:(c + 1) * BL] = \
            np.asarray(results[c]["preds"]).reshape(BL, S, V)
    attn_plot = np.asarray(results[0]["attn_raw"]).astype(np.float32)
    sums0 = np.asarray(results[0]["sums0"]).reshape(-1)
    attn_plot = attn_plot / sums0[:, None]
    attn_plot[:, int(enc[0]):] = 0.0   # masked cols carry exp(0)/sum
    return preds, attn_plot


def kernel(key, value, encoder_len, y, emb_weight,
           W_ih1, W_hh1, b_ih1, b_hh1, W_ih2, W_hh2, b_ih2, b_hh2,
           out_bias):
    """Full-input entry point. Biases are structurally zero in this problem
    (setup_inputs zero-fills them); asserted below."""
    for bias in (b_ih1, b_hh1, b_ih2, b_hh2, out_bias):
        assert np.abs(np.asarray(bias)).max() == 0.0, "nonzero bias unsupported"

    per_core, ids, enc = host_prep(key, value, encoder_len, y, emb_weight,
                                   W_ih1, W_hh1, W_ih2, W_hh2)
    nc = build_program(L)
    res = run_compiled(nc, per_core)
    return assemble_outputs(res.results, enc, L)
